# revision 18
# baseline (speedup 1.0000x reference)
"""Distributed sparse MoE (top-1 routing) kernel for 8 TRN2 NeuronCores.

Strategy (expert-parallel):
  - Shard tokens 1024/core for the router; each core computes logits->argmax/gate
    for its slice only (plain fp32 matmul for bit-safe argmax vs reference).
  - Each core writes a gate-scaled, gate-augmented copy of its token slice
    (x_aug = [gate*x, gate, pad]) and AllGathers it (plus the expert ids).
  - Core c owns expert c: selects the global token ids routed to expert c
    (sparse_gather stream compaction), dma_gathers those rows, transposes them
    on the PE, runs the expert GEMM (+ gate-scaled bias), and scatters result
    rows back to their global positions via dma_scatter_add.
  - Host combine: per-core outputs have disjoint nonzero rows -> sum.
"""

import sys

sys.path.insert(0, "/opt/trn_rl_repo")

import numpy as np

import concourse.bass as bass
import concourse.mybir as mybir
import concourse.tile as tile
from concourse import bacc
from concourse.bass_utils import run_bass_kernel_spmd
from concourse.masks import make_identity

F32 = mybir.dt.float32
F32R = mybir.dt.float32r
I16 = mybir.dt.int16
U32 = mybir.dt.uint32

N_CORES = 8
B, S, H, E = 4, 2048, 1024, 8
T = B * S                # 8192 tokens
TPC = T // N_CORES       # 1024 tokens per core slice
TILES = TPC // 128       # 8 token tiles per slice
HC = H // 128            # 8 contraction chunks
CAP = 1280               # per-expert token capacity (max actual ~1087)
CTIL = CAP // 128        # 10 gathered token tiles
AUGW = 1088              # 1024 + gate col + pad; 1088*4B = 4352 = 17*256
NHALF = 2                # 1024 output dims in 2 x 512 psum halves


def _body(tc, x, rw, rb, ew, eb, eid, iota1, slots, out, phases="abcgde"):
    nc = tc.nc
    P = 128
    Exp = mybir.ActivationFunctionType.Exp

    const = tc.alloc_tile_pool(name="const", bufs=1)
    ident = const.tile([P, P], F32)
    make_identity(nc, ident)

    rw_sb = const.tile([P, HC, E], F32)
    nc.sync.dma_start(rw_sb[:], rw.rearrange("(c p) e -> p c e", p=P))
    rb_sb = const.tile([1, E], F32)
    nc.sync.dma_start(rb_sb[:], rb[:])
    rb_rep = const.tile([P, E], F32)
    nc.gpsimd.partition_broadcast(rb_rep[:], rb_sb[:])

    w_sb = const.tile([P, HC, H], F32R)
    nc.sync.dma_start(w_sb[:], ew.rearrange("(c p) d -> p c d", p=P).bitcast(F32R))
    eb_sb = const.tile([1, H], F32)
    nc.sync.dma_start(eb_sb[:], eb[:])
    b_rep = const.tile([P, H], F32)
    nc.gpsimd.partition_broadcast(b_rep[:], eb_sb[:])

    eid_sb = const.tile([1, 1], F32)
    nc.sync.dma_start(eid_sb[:], eid[:])
    eid16 = const.tile([16, 1], F32)
    nc.gpsimd.partition_broadcast(eid16[:], eid_sb[:])

    iota1_sb = const.tile([16, T // 16], F32)
    nc.sync.dma_start(iota1_sb[:], iota1[:])
    slots_sb = const.tile([16, CAP // 16], F32)
    nc.sync.dma_start(slots_sb[:], slots[:])

    dram = tc.alloc_tile_pool(name="dram", bufs=1, space="DRAM")
    xaug_self = dram.tile([TPC, AUGW], F32)
    xaug_all = dram.tile([T, AUGW], F32, addr_space="Shared")

    # ---- Phase A: router on own slice + write scaled/augmented tokens ----
    with tc.tile_pool(name="workA", bufs=4) as workA, tc.tile_pool(
        name="psumA", bufs=2, space="PSUM"
    ) as psumA:
        for t in range(TILES):
            xt = workA.tile([P, H], F32, tag="xt")
            nc.sync.dma_start(xt[:], x[t * P : (t + 1) * P, :])
            xT = workA.tile([P, H], F32, tag="xT")
            pt = psumA.tile([P, H], F32, tag="pt")
            for c in range(HC):
                nc.tensor.transpose(
                    pt[:, c * P : (c + 1) * P], xt[:, c * P : (c + 1) * P], ident[:]
                )
            nc.scalar.copy(xT[:], pt[:])
            lp = psumA.tile([P, E], F32, tag="lp")
            for c in range(HC):
                nc.tensor.matmul(
                    lp[:],
                    lhsT=xT[:, c * P : (c + 1) * P],
                    rhs=rw_sb[:, c, :],
                    start=(c == 0),
                    stop=(c == HC - 1),
                )
            logits = workA.tile([P, E], F32, tag="logits")
            nc.vector.tensor_tensor(logits[:], lp[:], rb_rep[:], mybir.AluOpType.add)
            negmax = workA.tile([P, 1], F32, tag="negmax")
            nc.vector.reduce_max(
                negmax[:], logits[:], mybir.AxisListType.X, negate=True
            )
            expd = workA.tile([P, E], F32, tag="expd")
            esum = workA.tile([P, 1], F32, tag="esum")
            nc.scalar.activation(
                expd[:], logits[:], Exp, bias=negmax[:], accum_out=esum[:]
            )
            gate = workA.tile([P, 1], F32, tag="gate")
            nc.vector.reciprocal(gate[:], esum[:])
            mx8 = workA.tile([P, 8], F32, tag="mx8")
            nc.vector.max(mx8[:], logits[:])
            mi = workA.tile([P, 8], U32, tag="mi")
            nc.vector.max_index(mi[:], mx8[:], logits[:])
            xs = workA.tile([P, AUGW], F32, tag="xs")
            nc.vector.tensor_scalar_mul(xs[:, 0:H], xt[:], gate[:])
            nc.vector.tensor_copy(xs[:, H : H + 1], gate[:])
            nc.vector.tensor_copy(xs[:, H + 1 : H + 2], mi[:, 0:1])
            nc.vector.memset(xs[:, H + 2 : AUGW], 0.0)
            nc.sync.dma_start(xaug_self[t * P : (t + 1) * P, :], xs[:])

    # ---- Phase B: share router decisions + tokens ----
    if "b" not in phases:
        nc.sync.dma_start(out[0:128, 0:H], xaug_self[0:128, 0:H])
        dram.release()
        const.release()
        return
    rg = [list(range(N_CORES))]
    nc.gpsimd.collective_compute(
        "AllGather",
        mybir.AluOpType.bypass,
        replica_groups=rg,
        ins=[xaug_self[:].opt()],
        outs=[xaug_all[:].opt()],
    )

    if "c" not in phases:
        nc.sync.dma_start(out[0:8, :].rearrange("a b -> (a b)"), xaug_all[:, H + 1 : H + 2].rearrange("a one -> (a one)"))
        dram.release()
        const.release()
        return
    # ---- Phase C: select my expert's tokens, gather them ----
    sel = tc.alloc_tile_pool(name="sel", bufs=1)
    idx16 = sel.tile([16, T // 16], F32)
    nc.sync.dma_start(idx16[:], xaug_all[:, H + 1 : H + 2].rearrange("(f p) one -> p (f one)", p=16))
    eq = sel.tile([16, T // 16], F32)
    nc.vector.tensor_scalar(
        eq[:], idx16[:], eid16[:], None, op0=mybir.AluOpType.is_equal
    )
    val = sel.tile([16, T // 16], F32)
    nc.vector.tensor_tensor(val[:], iota1_sb[:], eq[:], mybir.AluOpType.mult)
    nc.vector.tensor_scalar_add(val[:], val[:], -1.0)

    stage = sel.tile([16, CAP // 16], F32)
    cnt = sel.tile([1, 1], U32)
    nc.gpsimd.sparse_gather(stage[:], val[:], num_found=cnt[:])

    cntf = sel.tile([1, 1], F32)
    nc.vector.tensor_copy(cntf[:], cnt[:])
    cnt16 = sel.tile([16, 1], F32)
    nc.gpsimd.partition_broadcast(cnt16[:], cntf[:])
    tailm = sel.tile([16, CAP // 16], F32)
    nc.vector.tensor_scalar(
        tailm[:], slots_sb[:], cnt16[:], None, op0=mybir.AluOpType.is_lt
    )
    # valid slots -> token id; tail slots -> T (out-of-bounds sentinel, skipped)
    fixed = sel.tile([16, CAP // 16], F32)
    nc.vector.tensor_scalar_add(fixed[:], stage[:], -float(T))
    nc.vector.tensor_tensor(fixed[:], fixed[:], tailm[:], mybir.AluOpType.mult)
    nc.vector.tensor_scalar_add(fixed[:], fixed[:], float(T))

    idx32w = sel.tile([16, CAP // 16], mybir.dt.int32)
    nc.vector.tensor_copy(idx32w[:], fixed[:])
    idx_flat = dram.tile([CAP], mybir.dt.int32)
    nc.sync.dma_start(idx_flat[:].rearrange("(f p) -> p f", p=16), idx32w[:])
    idxp = sel.tile([P, CTIL], mybir.dt.int32)
    nc.sync.dma_start(idxp[:], idx_flat[:].rearrange("(j p) -> p j", p=P))

    if "g" not in phases:
        nc.sync.dma_start(out[0:1, 0 : CAP // 16].rearrange("a b -> (a b)"), fixed[:].rearrange("a b -> (a b)")[0 : CAP // 16])
        sel.release()
        dram.release()
        const.release()
        return
    big = tc.alloc_tile_pool(name="big", bufs=1)
    gath = big.tile([P, CTIL, AUGW], F32)
    for j in range(CTIL):
        nc.gpsimd.indirect_dma_start(
            out=gath[:, j, :],
            out_offset=None,
            in_=xaug_all[:],
            in_offset=bass.IndirectOffsetOnAxis(ap=idxp[:, j : j + 1], axis=0),
            bounds_check=T - 1,
            oob_is_err=False,
        )

    if "d" not in phases:
        nc.sync.dma_start(out[0:128, 0:H], gath[:, 0, 0:H])
        big.release()
        sel.release()
        dram.release()
        const.release()
        return
    # ---- Phase D: expert GEMM on gathered tokens ----
    outsb = big.tile([P, CTIL, H], F32)
    with tc.tile_pool(name="workD", bufs=3) as workD, tc.tile_pool(
        name="psumT", bufs=2, space="PSUM"
    ) as psumT, tc.tile_pool(name="psumG", bufs=2, space="PSUM") as psumG:
        for j in range(CTIL):
            xTg = workD.tile([P, HC, P], F32R, tag="xTg")
            pt = psumT.tile([P, H], F32, tag="pt")
            for c in range(HC):
                nc.tensor.transpose(
                    pt[:, c * P : (c + 1) * P], gath[:, j, c * P : (c + 1) * P], ident[:]
                )
            nc.scalar.copy(xTg[:].rearrange("p c d -> p (c d)"), pt[:])
            gate_g = gath[:, j, H : H + 1]
            for h in range(NHALF):
                pg = psumG.tile([P, 512], F32, tag="pg")
                for c in range(HC):
                    nc.tensor.matmul(
                        pg[:],
                        lhsT=xTg[:, c, :],
                        rhs=w_sb[:, c, h * 512 : (h + 1) * 512],
                        start=(c == 0),
                        stop=(c == HC - 1),
                    )
                bg = workD.tile([P, 512], F32, tag="bg")
                nc.vector.tensor_scalar_mul(
                    bg[:], b_rep[:, h * 512 : (h + 1) * 512], gate_g
                )
                nc.vector.tensor_tensor(
                    outsb[:, j, h * 512 : (h + 1) * 512],
                    pg[:],
                    bg[:],
                    mybir.AluOpType.add,
                )

    if "e" not in phases:
        nc.sync.dma_start(out[0:128, :], outsb[:, 0, :])
        big.release()
        sel.release()
        dram.release()
        const.release()
        return
    # ---- Phase E: scatter rows back to global positions ----
    for j in range(CTIL):
        nc.gpsimd.indirect_dma_start(
            out=out[:],
            out_offset=bass.IndirectOffsetOnAxis(ap=idxp[:, j : j + 1], axis=0),
            in_=outsb[:, j, :],
            in_offset=None,
            bounds_check=T - 1,
            oob_is_err=False,
        )

    big.release()
    sel.release()
    dram.release()
    const.release()


def build_kernel(phases="abcgde"):
    nc = bacc.Bacc(
        "TRN2",
        target_bir_lowering=False,
        debug=False,
        enable_asserts=True,
        num_devices=N_CORES,
    )
    x = nc.dram_tensor("x", [TPC, H], F32, kind="ExternalInput").ap()
    rw = nc.dram_tensor("router_w", [H, E], F32, kind="ExternalInput").ap()
    rb = nc.dram_tensor("router_b", [1, E], F32, kind="ExternalInput").ap()
    ew = nc.dram_tensor("expert_w", [H, H], F32, kind="ExternalInput").ap()
    eb = nc.dram_tensor("expert_b", [1, H], F32, kind="ExternalInput").ap()
    eid = nc.dram_tensor("eid", [1, 1], F32, kind="ExternalInput").ap()
    iota1 = nc.dram_tensor("iota1", [16, T // 16], F32, kind="ExternalInput").ap()
    slots = nc.dram_tensor("slots", [16, CAP // 16], F32, kind="ExternalInput").ap()
    out = nc.dram_tensor("out", [T, H], F32, kind="ExternalOutput").ap()

    with tile.TileContext(nc) as tc:
        _body(tc, x, rw, rb, ew, eb, eid, iota1, slots, out, phases=phases)
    nc.compile()
    return nc


_CACHE = {}


def _wrap16(n, vals):
    """Values laid out so element k sits at [k % 16, k // 16]."""
    a = np.asarray(vals, dtype=np.float32)
    return a.reshape(-1, 16).T.copy()


def kernel(x, router_w, router_b, expert_w, expert_b, **run_kwargs):
    x = np.ascontiguousarray(np.asarray(x, dtype=np.float32))
    router_w = np.ascontiguousarray(np.asarray(router_w, dtype=np.float32))
    router_b = np.ascontiguousarray(np.asarray(router_b, dtype=np.float32))
    expert_w = np.ascontiguousarray(np.asarray(expert_w, dtype=np.float32))
    expert_b = np.ascontiguousarray(np.asarray(expert_b, dtype=np.float32))

    hs = x.reshape(T, H)
    iota1 = _wrap16(T, np.arange(1, T + 1, dtype=np.float32))
    slots = _wrap16(CAP, np.arange(CAP, dtype=np.float32))

    import os
    phases = os.environ.get("KPHASES", "abcgde")
    if _CACHE.get("phases") != phases:
        _CACHE["nc"] = build_kernel(phases)
        _CACHE["phases"] = phases
    nc = _CACHE["nc"]

    in_maps = []
    for c in range(N_CORES):
        in_maps.append(
            {
                "x": hs[c * TPC : (c + 1) * TPC],
                "router_w": router_w,
                "router_b": router_b.reshape(1, E),
                "expert_w": expert_w[c],
                "expert_b": expert_b[c].reshape(1, H),
                "eid": np.full((1, 1), float(c), dtype=np.float32),
                "iota1": iota1,
                "slots": slots,
            }
        )

    res = run_bass_kernel_spmd(nc, in_maps, core_ids=list(range(N_CORES)), **run_kwargs)
    full = np.zeros((T, H), dtype=np.float32)
    for r in res.results:
        full += r["out"]
    out = full.reshape(B, S, H)
    if run_kwargs:
        return out, res
    return out
